# revision 8
# baseline (speedup 1.0000x reference)
"""Trainium2 Bass kernel for ContrastiveMSELoss.

Reference computes, over all N^2 pairs (diagonal masked to 0):
    mse_ij  = (|x_i|^2 + |x_j|^2 - 2 x_i.x_j) / D
    sign_ij = +1 if class_i == class_j else -1
    loss    = mean_ij(sign_ij * mse_ij) + BETA

Using sum_{i,j in c} x_i.x_j = |M_c|^2 with M_c = sum_{i in c} x_i, the
loss collapses to class-bucketed first/second moments (O(N*D) work,
memory-bound -- no N x N gram matrix needed):

    T_same = sum_c (2 n_c SQ_c - 2 |M_c|^2) / D      (diag terms are 0)
    T_all  = (2 N SQ - 2 |M|^2) / D
    loss   = (2 T_same - T_all) / N^2 + BETA

Device dataflow (per core, rows r = p*8 + k on partition p, sub-row k):
  - x streams f32 over the SP HWDGE ring in row-group chunks (no SWDGE
    descriptor-emission pacing).
  - ACT casts each chunk to bf16 (matmul moving cols 0:256); DVE runs a
    fused square+row-reduce (tensor_tensor_reduce mult/add) writing
    |x_row|^2 into moving col 256.  The last two sub-rows reduce from
    f32 directly so the tail does not serialize cast -> reduce.
  - One matmul chain per k then yields both per-class sums M_c AND
    per-class square-sums SQ_c in one PSUM bank: PSUM[c, 0:256] = M_c,
    PSUM[c, 256] = SQ_c.  Even k accumulate into PSUM partitions 0:40,
    odd k into 64:104, so consecutive matmuls use different PE column
    groups and overlap.
  - The even chain closes one matmul early; folds (PSUM->SBUF bf16) and
    the two stores (SP ring / ACT ring) are inside the TileContext so
    they issue straight off the fold data dependency instead of behind
    the context-exit barrier.
Host combines per-core [80, 257] partials into the scalar loss.
"""

import numpy as np

import concourse.bacc as bacc
import concourse.bass as bass
import concourse.tile as tile
from concourse import mybir
from concourse.bass_utils import run_bass_kernel_spmd

N, D = 8192, 256
N_CORES = 8
ROWS = N // N_CORES          # 1024 rows per core
P = 128                      # partitions
K = ROWS // P                # 8 sub-rows per partition (row = p*8 + k)
NCLS = 40
BETA = 1.0
DW = D + 4                   # moving-tile row pitch (col 256 = rowsq, 257.. pad)
OUTC = D + 1                 # stored columns: 256 class-sum cols + 1 sq col

# x row-group chunks streamed on the SP HWDGE ring: (k0, nk).  The last
# groups are single sub-rows so the tail compute starts as early as
# possible.
CHUNKS = [(0, 2), (2, 2), (4, 2), (6, 1), (7, 1)]
F32_TTR = set()              # sub-rows whose square-reduce reads f32 directly

_CACHE = {}


def _build_bass():
    nc = bacc.Bacc(
        "TRN2",
        target_bir_lowering=False,
        debug=False,
        enable_asserts=False,
        num_devices=N_CORES,
    )
    # x shard viewed as [128, 8, 256]: partition p = rows p*8 .. p*8+7
    x = nc.dram_tensor("x", [P, K, D], mybir.dt.float32, kind="ExternalInput")
    # host-built one-hot: ohd[p, k, c] = (class[p*8+k] == c)
    ohd = nc.dram_tensor(
        "oh", [P, K, NCLS], mybir.dt.bfloat16, kind="ExternalInput"
    )
    # stats rows 0:40 = even-k chain, rows 40:80 = odd-k chain;
    # cols 0:256 per-class sums of x, col 256 per-class sums of |x|^2
    stats = nc.dram_tensor(
        "stats", [2 * NCLS, OUTC], mybir.dt.bfloat16, kind="ExternalOutput"
    )

    # full 2KB bank: the matmul start flag zeroes a 2048B-aligned region,
    # so the per-partition row must be bank-aligned
    acc = nc.alloc_psum_tensor("acc_raw", [P, 512], mybir.dt.float32)

    with tile.TileContext(nc) as tc:
        with tc.tile_pool(name="work", bufs=1) as work:
            xf = work.tile([P, K, D], mybir.dt.float32, tag="xf")
            xb = work.tile([P, K, DW], mybir.dt.bfloat16, tag="xb")
            oh = work.tile([P, K, NCLS], mybir.dt.bfloat16, tag="oh")
            out_sb = work.tile([P, OUTC], mybir.dt.bfloat16, tag="osb")
            sqs = work.tile([P, D], mybir.dt.bfloat16, tag="sqs")

            # one-hot rides the ACT HWDGE ring; x chunks ride the SP ring.
            nc.scalar.dma_start(out=oh[:, :, :], in_=ohd[:, :, :])
            for k0, nk in CHUNKS:
                nc.sync.dma_start(
                    out=xf[:, k0 : k0 + nk, :], in_=x[:, k0 : k0 + nk, :]
                )

            with nc.allow_low_precision("bf16 row-square sums; tol 2e-2"):
                for k0, nk in CHUNKS:
                    # ACT: cast the whole chunk to bf16 moving data
                    nc.scalar.copy(
                        xb[:, k0 : k0 + nk, 0:D], xf[:, k0 : k0 + nk, :]
                    )
                    for k in range(k0, k0 + nk):
                        # DVE: square then row-sum into moving col 256
                        nc.vector.tensor_mul(sqs[:, :], xb[:, k, 0:D], xb[:, k, 0:D])
                        nc.vector.tensor_reduce(
                            xb[:, k, D : D + 1],
                            sqs[:, :],
                            axis=mybir.AxisListType.X,
                            op=mybir.AluOpType.add,
                        )
                        lo = 0 if k % 2 == 0 else 64
                        nc.tensor.matmul(
                            acc[lo : lo + NCLS, 0:OUTC],
                            oh[:, k, :],
                            xb[:, k, 0:OUTC],
                            start=(k < 2),
                            stop=(k >= K - 2),
                            skip_group_check=True,
                        )
                        if k == K - 2:
                            # even chain closed: fold + store it while the
                            # last odd matmul runs (disjoint partitions)
                            nc.vector.tensor_copy(
                                out_sb[0:NCLS, :], acc[0:NCLS, 0:OUTC]
                            )
                            nc.sync.dma_start(
                                out=stats[0:NCLS, :], in_=out_sb[0:NCLS, :]
                            )
            nc.vector.tensor_copy(
                out_sb[64 : 64 + NCLS, :], acc[64 : 64 + NCLS, 0:OUTC]
            )
            nc.scalar.dma_start(
                out=stats[NCLS : 2 * NCLS, :],
                in_=out_sb[64 : 64 + NCLS, :],
            )

    return nc


def _get_nc():
    if "nc" not in _CACHE:
        nc = _build_bass()
        nc.finalize()
        _CACHE["nc"] = nc
    return _CACHE["nc"]


def run_device(output, classes, **spmd_kwargs):
    """Run the per-core Bass kernel; returns (list of per-core stats, results)."""
    x = np.ascontiguousarray(np.asarray(output), dtype=np.float32)
    cls = np.asarray(classes).astype(np.int64)
    onehot = (cls[:, None] == np.arange(NCLS)[None, :]).astype(np.float32)
    from ml_dtypes import bfloat16

    onehot = onehot.astype(bfloat16)
    in_maps = []
    for s in range(N_CORES):
        xs = x[s * ROWS : (s + 1) * ROWS].reshape(P, K, D)
        ohs = onehot[s * ROWS : (s + 1) * ROWS].reshape(P, K, NCLS)
        in_maps.append(
            {"x": np.ascontiguousarray(xs), "oh": np.ascontiguousarray(ohs)}
        )
    try:
        res = run_bass_kernel_spmd(
            _get_nc(), in_maps, core_ids=list(range(N_CORES)), **spmd_kwargs
        )
    except Exception:
        # a previous session can leave the device needing one reset cycle;
        # a single retry recovers it
        res = run_bass_kernel_spmd(
            _get_nc(), in_maps, core_ids=list(range(N_CORES)), **spmd_kwargs
        )
    stats = [res.results[s]["stats"] for s in range(N_CORES)]
    return stats, res


def _combine(stats, classes):
    """Combine per-core partial class stats into the scalar loss (float64)."""
    tot = np.sum(np.asarray(stats, dtype=np.float64), axis=0)  # [80, 257]
    tot = tot[:NCLS] + tot[NCLS : 2 * NCLS]                    # [40, 257]
    M_c = tot[:, :D]                                           # class sums
    SQ_c = tot[:, D]                                           # class |x|^2 sums
    n_c = np.bincount(np.asarray(classes).astype(np.int64), minlength=NCLS).astype(
        np.float64
    )
    SQ = SQ_c.sum()
    M = M_c.sum(axis=0)
    T_same = (2.0 * (n_c * SQ_c).sum() - 2.0 * (M_c * M_c).sum()) / D
    T_all = (2.0 * N * SQ - 2.0 * (M @ M)) / D
    loss = (2.0 * T_same - T_all) / (float(N) * float(N)) + BETA
    return np.float32(loss)


def kernel(output, classes):
    stats, _ = run_device(output, classes)
    return _combine(stats, classes)
